# revision 2
# baseline (speedup 1.0000x reference)
"""Trainium2 Bass kernel for the CorpBEVT fused gather-scatter.

Reference semantics (B=1, L=n=5, C=128, H*W=65536, K=32768):
    out[n, c, hw] = x[0, n, c, hw]             if hw in selected_indices
                    orig_bev[ego_index, c, hw]  otherwise
    returned as [5, 128, 256, 256] float32.

This is a pure elementwise select between x and the (replicated) ego BEV,
with the predicate depending only on the spatial position hw. The indices
are host-visible, so we precompute a uint8 "not selected" mask on the host
and the device kernel is a DMA-bound streaming select:

  - shard hw (65536) across the 8 NeuronCores -> 8192 columns per core
  - per core: keep the ego slab [128, 8192] and the inverse mask resident
    in SBUF, stream x[n] tiles in, one DVE copy_predicated overwrites the
    not-selected lanes with ego, stream the tile out.

Per-core HBM traffic: 20 MB x-in + 4 MB ego + 1 MB mask + 20 MB out
~= 45 MB -> ~130 us at the ~358 GB/s HBM-per-core roofline.
"""

import sys

if "/opt/trn_rl_repo" not in sys.path:
    sys.path.insert(0, "/opt/trn_rl_repo")

import numpy as np

import concourse.bacc as bacc
import concourse.mybir as mybir
from concourse import tile
from concourse.bass_utils import run_bass_kernel_spmd

N_CORES = 8
N, C, H, W = 5, 128, 256, 256
HW = H * W             # 65536
SHARD = HW // N_CORES  # 8192 columns per core

CHUNK = 8192       # columns per streamed tile
STREAM_BUFS = 3    # x-tile slots (load / compute / store overlap)
CONST_BUFS = 2     # ego+mask slots (lets bench reps overlap const reload)
BENCH_UNROLL = 4

_NC_CACHE = {}


def _build_nc(bench_repeat=0, chunk=CHUNK, stream_bufs=STREAM_BUFS):
    """Build + compile the per-core Bass program (identical on all cores).

    bench_repeat=0: the graded kernel — external I/O, body runs once.
    bench_repeat>0: timing variant — body repeated bench_repeat times over
        *Internal* (device-resident, uninitialized) DRAM so a timed call
        uploads/downloads only a dummy scalar. Timing is data-independent
        (pure DMA + predicated copy), so garbage contents are fine.
    """
    assert SHARD % chunk == 0
    nc = bacc.Bacc("TRN2", target_bir_lowering=False, debug=False)
    f32 = mybir.dt.float32
    u8 = mybir.dt.uint8

    bench = bench_repeat > 0
    io_kind = {} if bench else {"kind": "ExternalInput"}
    out_kind = {} if bench else {"kind": "ExternalOutput"}
    x_d = nc.dram_tensor("xs", [N, C, SHARD], f32, **io_kind)
    ego_d = nc.dram_tensor("egos", [C, SHARD], f32, **io_kind)
    m_d = nc.dram_tensor("invmask", [C, SHARD], u8, **io_kind)
    out_d = nc.dram_tensor("outs", [N, C, SHARD], f32, **out_kind)
    if bench:
        dummy_in = nc.dram_tensor("dummy_in", [1, 1], f32, kind="ExternalInput")
        dummy_out = nc.dram_tensor("dummy_out", [1, 1], f32, kind="ExternalOutput")

    with tile.TileContext(nc) as tc:
        with (
            tc.tile_pool(name="const", bufs=CONST_BUFS) as cpool,
            tc.tile_pool(name="stream", bufs=stream_bufs) as spool,
        ):

            def full_pass():
                ego_t = cpool.tile([C, SHARD], f32, tag="ego")
                m_t = cpool.tile([C, SHARD], u8, tag="mask")
                nc.sync.dma_start(ego_t[:], ego_d[:])
                nc.sync.dma_start(m_t[:], m_d[:])
                for n in range(N):
                    for j in range(SHARD // chunk):
                        cs = slice(j * chunk, (j + 1) * chunk)
                        x_t = spool.tile([C, chunk], f32, tag="x")
                        nc.sync.dma_start(x_t[:], x_d[n, :, cs])
                        # overwrite not-selected lanes of x with ego
                        nc.vector.copy_predicated(
                            x_t[:], m_t[:, cs], ego_t[:, cs]
                        )
                        nc.sync.dma_start(out_d[n, :, cs], x_t[:])

            if bench:
                d_t = cpool.tile([1, 1], f32, tag="dummy")
                nc.sync.dma_start(d_t[:], dummy_in[:])
                nc.sync.dma_start(dummy_out[:], d_t[:])
                assert bench_repeat % BENCH_UNROLL == 0
                with tc.For_i(0, bench_repeat // BENCH_UNROLL, 1):
                    for _ in range(BENCH_UNROLL):
                        full_pass()
            else:
                full_pass()

    nc.compile()
    return nc


def _get_nc(bench_repeat=0, **kwargs):
    key = (bench_repeat, tuple(sorted(kwargs.items())))
    if key not in _NC_CACHE:
        _NC_CACHE[key] = _build_nc(bench_repeat, **kwargs)
    return _NC_CACHE[key]


def _make_in_maps(x, orig_bev, selected_indices, ego_index):
    x = np.asarray(x, dtype=np.float32)
    orig_bev = np.asarray(orig_bev, dtype=np.float32)
    idx = np.asarray(selected_indices).astype(np.int64, copy=False)

    x_flat = x.reshape(N, C, HW)
    ego_flat = orig_bev[int(ego_index)].reshape(C, HW)

    inv = np.ones(HW, dtype=np.uint8)
    inv[idx] = 0

    in_maps = []
    for core in range(N_CORES):
        s = core * SHARD
        e = s + SHARD
        in_maps.append(
            {
                "xs": np.ascontiguousarray(x_flat[:, :, s:e]),
                "egos": np.ascontiguousarray(ego_flat[:, s:e]),
                "invmask": np.ascontiguousarray(
                    np.broadcast_to(inv[s:e], (C, SHARD))
                ),
            }
        )
    return in_maps


def _run(x, orig_bev, selected_indices, ego_index, **spmd_kwargs):
    """Shared entry for kernel() and the harness in test.py."""
    nc = _get_nc()
    in_maps = _make_in_maps(x, orig_bev, selected_indices, ego_index)
    res = run_bass_kernel_spmd(
        nc, in_maps, core_ids=list(range(N_CORES)), **spmd_kwargs
    )
    out = np.concatenate(
        [np.asarray(res.results[c]["outs"]) for c in range(N_CORES)], axis=2
    )
    return out.reshape(N, C, H, W).astype(np.float32, copy=False), res


def kernel(x, orig_bev, selected_indices, ego_index):
    out, _ = _run(x, orig_bev, selected_indices, ego_index)
    return out


def bench_run(bench_repeat, **build_kwargs):
    """One timed execution of the bench variant; returns wallclock seconds."""
    import time

    nc = _get_nc(bench_repeat, **build_kwargs)
    in_maps = [{"dummy_in": np.zeros((1, 1), np.float32)} for _ in range(N_CORES)]
    t0 = time.time()
    run_bass_kernel_spmd(nc, in_maps, core_ids=list(range(N_CORES)))
    return time.time() - t0


# revision 10
# speedup vs baseline: 1.1996x; 1.1996x over previous
"""Trainium2 Bass kernel for the CorpBEVT fused gather-scatter.

Reference semantics (B=1, L=n=5, C=128, H*W=65536, K=32768):
    out[n, c, hw] = x[0, n, c, hw]             if hw in selected_indices
                    orig_bev[ego_index, c, hw]  otherwise
    returned as [5, 128, 256, 256] float32.

This is a pure elementwise select between x and the (replicated) ego BEV,
with the predicate depending only on the spatial position hw. The indices
are host-visible, so we precompute a uint8 "not selected" mask on the host
and the device kernel is a DMA-bound streaming select:

  - shard hw (65536) across the 8 NeuronCores -> 8192 columns per core
  - per core: keep the ego slab [128, 8192] and the inverse mask resident
    in SBUF, stream x[n] tiles in, one DVE copy_predicated overwrites the
    not-selected lanes with ego, stream the tile out.

Per-core HBM traffic: 20 MB x-in + 4 MB ego + mask + 20 MB out
~= 45 MB -> ~130 us at the ~358 GB/s HBM-per-core roofline.
"""

import sys

if "/opt/trn_rl_repo" not in sys.path:
    sys.path.insert(0, "/opt/trn_rl_repo")

import numpy as np

import concourse.bacc as bacc
import concourse.mybir as mybir
from concourse import tile
from concourse.bass_utils import run_bass_kernel_spmd

N_CORES = 8
N, C, H, W = 5, 128, 256, 256
HW = H * W             # 65536
SHARD = HW // N_CORES  # 8192 columns per core

# Tuning knobs (best known configuration; see test.py sweeps).
CHUNK = 8192         # columns per streamed tile (nmajor layout)
STREAM_BUFS = 3      # x-tile slots (load / compute / store overlap)
CONST_BUFS = 2       # ego+mask slots (lets bench reps overlap const reload)
SPLIT_RINGS = False  # one HWDGE ring measured faster than two
BCAST_MASK = True    # upload mask as [1, SHARD]; broadcast on device
LAYOUT = "nmajor"    # "nmajor": x slab [N,C,SHARD]; "cmajor": [C, N*SHARD]
BENCH_UNROLL = 8

# cmajor chunking: slab-aligned chunks of the [C, N*SHARD] view, in columns.
CM_CHUNKS = (2 * SHARD, 2 * SHARD, SHARD)  # 8 MB, 8 MB, 4 MB transfers

_NC_CACHE = {}


def _build_nc(
    bench_repeat=0,
    chunk=CHUNK,
    stream_bufs=STREAM_BUFS,
    const_bufs=CONST_BUFS,
    split_rings=SPLIT_RINGS,
    bcast_mask=BCAST_MASK,
    layout=LAYOUT,
    cm_chunks=CM_CHUNKS,
    const_ring="sync",
    store_ring="sync",
    unroll=BENCH_UNROLL,
    no_compute=False,
):
    """Build + compile the per-core Bass program (identical on all cores).

    bench_repeat=0: the graded kernel — external I/O, body runs once.
    bench_repeat>0: timing variant — body repeated bench_repeat times over
        *Internal* (device-resident, uninitialized) DRAM so a timed call
        uploads/downloads only a dummy scalar. Timing is data-independent
        (pure DMA + predicated copy), so garbage contents are fine.
    no_compute: bench-only — drop the copy_predicated ops to measure the
        pure-DMA floor.
    """
    assert SHARD % chunk == 0
    nc = bacc.Bacc("TRN2", target_bir_lowering=False, debug=False)
    f32 = mybir.dt.float32
    u8 = mybir.dt.uint8

    bench = bench_repeat > 0
    io_kind = {} if bench else {"kind": "ExternalInput"}
    out_kind = {} if bench else {"kind": "ExternalOutput"}
    cmajor = layout == "cmajor"
    if cmajor:
        assert sum(cm_chunks) == N * SHARD
        assert all(c % SHARD == 0 for c in cm_chunks)
        x_shape = out_shape = [C, N * SHARD]
    else:
        x_shape = out_shape = [N, C, SHARD]
    x_d = nc.dram_tensor("xs", x_shape, f32, **io_kind)
    ego_d = nc.dram_tensor("egos", [C, SHARD], f32, **io_kind)
    mask_shape = [1, SHARD] if bcast_mask else [C, SHARD]
    m_d = nc.dram_tensor("invmask", mask_shape, u8, **io_kind)
    out_d = nc.dram_tensor("outs", out_shape, f32, **out_kind)
    if bench:
        dummy_in = nc.dram_tensor("dummy_in", [1, 1], f32, kind="ExternalInput")
        dummy_out = nc.dram_tensor("dummy_out", [1, 1], f32, kind="ExternalOutput")

    load_eng = nc.sync
    rings = {"sync": nc.sync, "act": nc.scalar, "gpsimd": nc.gpsimd}
    store_eng = rings["act"] if split_rings else rings[store_ring]
    const_eng = rings["act"] if const_ring == "act" else store_eng

    with tile.TileContext(nc) as tc:
        with (
            tc.tile_pool(name="const", bufs=const_bufs) as cpool,
            tc.tile_pool(name="stream", bufs=stream_bufs) as spool,
        ):

            def full_pass():
                ego_t = cpool.tile([C, SHARD], f32, tag="ego")
                m_t = cpool.tile([C, SHARD], u8, tag="mask")
                const_eng.dma_start(ego_t[:], ego_d[:])
                if bcast_mask:
                    m_row = cpool.tile([1, SHARD], u8, tag="maskrow")
                    const_eng.dma_start(m_row[:], m_d[:])
                    nc.gpsimd.partition_broadcast(m_t[:], m_row[:])
                else:
                    const_eng.dma_start(m_t[:], m_d[:])
                if cmajor:
                    col = 0
                    for ch in cm_chunks:
                        cs = slice(col, col + ch)
                        x_t = spool.tile([C, max(cm_chunks)], f32, tag="x")
                        load_eng.dma_start(x_t[:, :ch], x_d[:, cs])
                        if not no_compute:
                            # every SHARD-wide segment selects against the
                            # same full ego/mask slab
                            for k in range(ch // SHARD):
                                seg = slice(k * SHARD, (k + 1) * SHARD)
                                nc.vector.copy_predicated(
                                    x_t[:, seg], m_t[:], ego_t[:]
                                )
                        store_eng.dma_start(out_d[:, cs], x_t[:, :ch])
                        col += ch
                    return
                for n in range(N):
                    for j in range(SHARD // chunk):
                        cs = slice(j * chunk, (j + 1) * chunk)
                        x_t = spool.tile([C, chunk], f32, tag="x")
                        load_eng.dma_start(x_t[:], x_d[n, :, cs])
                        if not no_compute:
                            # overwrite not-selected lanes of x with ego
                            nc.vector.copy_predicated(
                                x_t[:], m_t[:, cs], ego_t[:, cs]
                            )
                        store_eng.dma_start(out_d[n, :, cs], x_t[:])

            if bench:
                d_t = cpool.tile([1, 1], f32, tag="dummy")
                nc.sync.dma_start(d_t[:], dummy_in[:])
                nc.sync.dma_start(dummy_out[:], d_t[:])
                assert bench_repeat % unroll == 0
                with tc.For_i(0, bench_repeat // unroll, 1):
                    for _ in range(unroll):
                        full_pass()
            else:
                full_pass()

    nc.compile()
    return nc


def _get_nc(bench_repeat=0, **kwargs):
    key = (bench_repeat, tuple(sorted(kwargs.items())))
    if key not in _NC_CACHE:
        _NC_CACHE[key] = _build_nc(bench_repeat, **kwargs)
    return _NC_CACHE[key]


def _make_in_maps(
    x, orig_bev, selected_indices, ego_index,
    bcast_mask=BCAST_MASK, layout=LAYOUT,
):
    x = np.asarray(x, dtype=np.float32)
    orig_bev = np.asarray(orig_bev, dtype=np.float32)
    idx = np.asarray(selected_indices).astype(np.int64, copy=False)

    x_flat = x.reshape(N, C, HW)
    ego_flat = orig_bev[int(ego_index)].reshape(C, HW)

    inv = np.ones(HW, dtype=np.uint8)
    inv[idx] = 0

    in_maps = []
    for core in range(N_CORES):
        s = core * SHARD
        e = s + SHARD
        if bcast_mask:
            m = inv[s:e].reshape(1, SHARD)
        else:
            m = np.ascontiguousarray(np.broadcast_to(inv[s:e], (C, SHARD)))
        xs = x_flat[:, :, s:e]
        if layout == "cmajor":
            # [N, C, SHARD] -> [C, N*SHARD]
            xs = xs.transpose(1, 0, 2).reshape(C, N * SHARD)
        in_maps.append(
            {
                "xs": np.ascontiguousarray(xs),
                "egos": np.ascontiguousarray(ego_flat[:, s:e]),
                "invmask": m,
            }
        )
    return in_maps


def _run(x, orig_bev, selected_indices, ego_index, **spmd_kwargs):
    """Shared entry for kernel() and the harness in test.py."""
    nc = _get_nc()
    in_maps = _make_in_maps(x, orig_bev, selected_indices, ego_index)
    res = run_bass_kernel_spmd(
        nc, in_maps, core_ids=list(range(N_CORES)), **spmd_kwargs
    )
    outs = [np.asarray(res.results[c]["outs"]) for c in range(N_CORES)]
    if LAYOUT == "cmajor":
        # [C, N*SHARD] -> [N, C, SHARD]
        outs = [o.reshape(C, N, SHARD).transpose(1, 0, 2) for o in outs]
    out = np.concatenate(outs, axis=2)
    return out.reshape(N, C, H, W).astype(np.float32, copy=False), res


def kernel(x, orig_bev, selected_indices, ego_index):
    out, _ = _run(x, orig_bev, selected_indices, ego_index)
    return out


def bench_run(bench_repeat, **build_kwargs):
    """One timed execution of the bench variant; returns wallclock seconds."""
    import time

    nc = _get_nc(bench_repeat, **build_kwargs)
    in_maps = [{"dummy_in": np.zeros((1, 1), np.float32)} for _ in range(N_CORES)]
    t0 = time.time()
    run_bass_kernel_spmd(nc, in_maps, core_ids=list(range(N_CORES)))
    return time.time() - t0


# revision 13
# speedup vs baseline: 1.2501x; 1.0421x over previous
"""Trainium2 Bass kernel for the CorpBEVT fused gather-scatter.

Reference semantics (B=1, L=n=5, C=128, H*W=65536, K=32768):
    out[n, c, hw] = x[0, n, c, hw]             if hw in selected_indices
                    orig_bev[ego_index, c, hw]  otherwise
    returned as [5, 128, 256, 256] float32.

This is a pure elementwise select between x and the (replicated) ego BEV,
with the predicate depending only on the spatial position hw. The indices
are host-visible, so we precompute a uint8 "not selected" mask on the host
and the device kernel is a DMA-bound streaming select:

  - shard hw (65536) across the 8 NeuronCores -> 8192 columns per core
  - per core: keep the ego slab [128, 8192] and the inverse mask resident
    in SBUF, stream x[n] tiles in, one DVE copy_predicated overwrites the
    not-selected lanes with ego, stream the tile out.

Per-core HBM traffic: 20 MB x-in + 4 MB ego + mask + 20 MB out
~= 45 MB -> ~130 us at the ~358 GB/s HBM-per-core roofline.
"""

import sys

if "/opt/trn_rl_repo" not in sys.path:
    sys.path.insert(0, "/opt/trn_rl_repo")

import numpy as np

import concourse.bacc as bacc
import concourse.mybir as mybir
from concourse import tile
from concourse.bass_utils import run_bass_kernel_spmd

N_CORES = 8
N, C, H, W = 5, 128, 256, 256
HW = H * W             # 65536
SHARD = HW // N_CORES  # 8192 columns per core

# Tuning knobs (best known configuration; see test.py sweeps).
CHUNK = 8192         # columns per streamed tile (nmajor layout)
STREAM_BUFS = 3      # x-tile slots (load / compute / store overlap)
CONST_BUFS = 2       # ego+mask slots (lets bench reps overlap const reload)
SPLIT_RINGS = False  # one HWDGE ring measured faster than two
BCAST_MASK = True    # upload mask as [1, SHARD]; broadcast on device
LAYOUT = "nmajor"    # "nmajor": x slab [N,C,SHARD]; "cmajor": [C, N*SHARD]
BENCH_UNROLL = 8

# cmajor chunking: slab-aligned chunks of the [C, N*SHARD] view, in columns.
CM_CHUNKS = (2 * SHARD, 2 * SHARD, SHARD)  # 8 MB, 8 MB, 4 MB transfers

_NC_CACHE = {}


def _build_nc(
    bench_repeat=0,
    chunk=CHUNK,
    stream_bufs=STREAM_BUFS,
    const_bufs=CONST_BUFS,
    split_rings=SPLIT_RINGS,
    bcast_mask=BCAST_MASK,
    layout=LAYOUT,
    cm_chunks=CM_CHUNKS,
    const_ring="sync",
    store_ring="sync",
    unroll=BENCH_UNROLL,
    no_compute=False,
    body_mode="full",
):
    """Build + compile the per-core Bass program (identical on all cores).

    bench_repeat=0: the graded kernel — external I/O, body runs once.
    bench_repeat>0: timing variant — body repeated bench_repeat times over
        *Internal* (device-resident, uninitialized) DRAM so a timed call
        uploads/downloads only a dummy scalar. Timing is data-independent
        (pure DMA + predicated copy), so garbage contents are fine.
    no_compute: bench-only — drop the copy_predicated ops to measure the
        pure-DMA floor.
    """
    assert SHARD % chunk == 0
    nc = bacc.Bacc("TRN2", target_bir_lowering=False, debug=False)
    f32 = mybir.dt.float32
    u8 = mybir.dt.uint8

    bench = bench_repeat > 0
    io_kind = {} if bench else {"kind": "ExternalInput"}
    out_kind = {} if bench else {"kind": "ExternalOutput"}
    cmajor = layout == "cmajor"
    if cmajor:
        assert sum(cm_chunks) == N * SHARD
        assert all(c % SHARD == 0 for c in cm_chunks)
        x_shape = out_shape = [C, N * SHARD]
    else:
        x_shape = out_shape = [N, C, SHARD]
    x_d = nc.dram_tensor("xs", x_shape, f32, **io_kind)
    ego_d = nc.dram_tensor("egos", [C, SHARD], f32, **io_kind)
    mask_shape = [1, SHARD] if bcast_mask else [C, SHARD]
    m_d = nc.dram_tensor("invmask", mask_shape, u8, **io_kind)
    out_d = nc.dram_tensor("outs", out_shape, f32, **out_kind)
    if bench:
        dummy_in = nc.dram_tensor("dummy_in", [1, 1], f32, kind="ExternalInput")
        dummy_out = nc.dram_tensor("dummy_out", [1, 1], f32, kind="ExternalOutput")

    load_eng = nc.sync
    rings = {"sync": nc.sync, "act": nc.scalar, "gpsimd": nc.gpsimd}
    store_eng = rings["act"] if split_rings else rings[store_ring]
    const_eng = rings["act"] if const_ring == "act" else store_eng

    with tile.TileContext(nc) as tc:
        with (
            tc.tile_pool(name="const", bufs=const_bufs) as cpool,
            tc.tile_pool(name="stream", bufs=stream_bufs) as spool,
        ):

            def full_pass():
                ego_t = cpool.tile([C, SHARD], f32, tag="ego")
                m_t = cpool.tile([C, SHARD], u8, tag="mask")
                const_eng.dma_start(ego_t[:], ego_d[:])
                if bcast_mask:
                    m_row = cpool.tile([1, SHARD], u8, tag="maskrow")
                    const_eng.dma_start(m_row[:], m_d[:])
                    nc.gpsimd.partition_broadcast(m_t[:], m_row[:])
                else:
                    const_eng.dma_start(m_t[:], m_d[:])
                if cmajor:
                    col = 0
                    for ch in cm_chunks:
                        cs = slice(col, col + ch)
                        x_t = spool.tile([C, max(cm_chunks)], f32, tag="x")
                        load_eng.dma_start(x_t[:, :ch], x_d[:, cs])
                        if not no_compute:
                            # every SHARD-wide segment selects against the
                            # same full ego/mask slab
                            for k in range(ch // SHARD):
                                seg = slice(k * SHARD, (k + 1) * SHARD)
                                nc.vector.copy_predicated(
                                    x_t[:, seg], m_t[:], ego_t[:]
                                )
                        store_eng.dma_start(out_d[:, cs], x_t[:, :ch])
                        col += ch
                    return
                if body_mode == "paired":
                    # batch same-direction DMAs pairwise: L,L,C,C,S,S
                    tiles = {}
                    for n in range(N):
                        tiles[n] = spool.tile([C, chunk], f32, tag="x", name=f"xp{n}")
                        load_eng.dma_start(tiles[n][:], x_d[n])
                        if n % 2 == 1 or n == N - 1:
                            grp = [n - 1, n] if n % 2 == 1 else [n]
                            for g in grp:
                                if not no_compute:
                                    nc.vector.copy_predicated(
                                        tiles[g][:], m_t[:], ego_t[:]
                                    )
                            for g in grp:
                                store_eng.dma_start(out_d[g], tiles[g][:])
                    return
                for n in range(N):
                    for j in range(SHARD // chunk):
                        cs = slice(j * chunk, (j + 1) * chunk)
                        if body_mode == "stores_only":
                            store_eng.dma_start(out_d[n, :, cs], ego_t[:, cs])
                            continue
                        x_t = spool.tile([C, chunk], f32, tag="x")
                        load_eng.dma_start(x_t[:], x_d[n, :, cs])
                        if body_mode == "loads_only":
                            continue
                        if not no_compute and body_mode == "full":
                            # overwrite not-selected lanes of x with ego
                            nc.vector.copy_predicated(
                                x_t[:], m_t[:, cs], ego_t[:, cs]
                            )
                        store_eng.dma_start(out_d[n, :, cs], x_t[:])

            if bench:
                d_t = cpool.tile([1, 1], f32, tag="dummy")
                nc.sync.dma_start(d_t[:], dummy_in[:])
                nc.sync.dma_start(dummy_out[:], d_t[:])
                assert bench_repeat % unroll == 0
                with tc.For_i(0, bench_repeat // unroll, 1):
                    for _ in range(unroll):
                        full_pass()
            else:
                full_pass()

    nc.compile()
    return nc


def _get_nc(bench_repeat=0, **kwargs):
    key = (bench_repeat, tuple(sorted(kwargs.items())))
    if key not in _NC_CACHE:
        _NC_CACHE[key] = _build_nc(bench_repeat, **kwargs)
    return _NC_CACHE[key]


def _make_in_maps(
    x, orig_bev, selected_indices, ego_index,
    bcast_mask=BCAST_MASK, layout=LAYOUT,
):
    x = np.asarray(x, dtype=np.float32)
    orig_bev = np.asarray(orig_bev, dtype=np.float32)
    idx = np.asarray(selected_indices).astype(np.int64, copy=False)

    x_flat = x.reshape(N, C, HW)
    ego_flat = orig_bev[int(ego_index)].reshape(C, HW)

    inv = np.ones(HW, dtype=np.uint8)
    inv[idx] = 0

    in_maps = []
    for core in range(N_CORES):
        s = core * SHARD
        e = s + SHARD
        if bcast_mask:
            m = inv[s:e].reshape(1, SHARD)
        else:
            m = np.ascontiguousarray(np.broadcast_to(inv[s:e], (C, SHARD)))
        xs = x_flat[:, :, s:e]
        if layout == "cmajor":
            # [N, C, SHARD] -> [C, N*SHARD]
            xs = xs.transpose(1, 0, 2).reshape(C, N * SHARD)
        in_maps.append(
            {
                "xs": np.ascontiguousarray(xs),
                "egos": np.ascontiguousarray(ego_flat[:, s:e]),
                "invmask": m,
            }
        )
    return in_maps


def _run(x, orig_bev, selected_indices, ego_index, **spmd_kwargs):
    """Shared entry for kernel() and the harness in test.py."""
    nc = _get_nc()
    in_maps = _make_in_maps(x, orig_bev, selected_indices, ego_index)
    res = run_bass_kernel_spmd(
        nc, in_maps, core_ids=list(range(N_CORES)), **spmd_kwargs
    )
    outs = [np.asarray(res.results[c]["outs"]) for c in range(N_CORES)]
    if LAYOUT == "cmajor":
        # [C, N*SHARD] -> [N, C, SHARD]
        outs = [o.reshape(C, N, SHARD).transpose(1, 0, 2) for o in outs]
    out = np.concatenate(outs, axis=2)
    return out.reshape(N, C, H, W).astype(np.float32, copy=False), res


def kernel(x, orig_bev, selected_indices, ego_index):
    out, _ = _run(x, orig_bev, selected_indices, ego_index)
    return out


def bench_run(bench_repeat, **build_kwargs):
    """One timed execution of the bench variant; returns wallclock seconds."""
    import time

    nc = _get_nc(bench_repeat, **build_kwargs)
    in_maps = [{"dummy_in": np.zeros((1, 1), np.float32)} for _ in range(N_CORES)]
    t0 = time.time()
    run_bass_kernel_spmd(nc, in_maps, core_ids=list(range(N_CORES)))
    return time.time() - t0
